# revision 13
# baseline (speedup 1.0000x reference)
"""GTN (Graph Transformer Network) kernel on 8 TRN2 NeuronCores via Bass/Tile.

Problem nn_GTN_17162689314910:
  A: [E=5, N=2048, N] f32, X: [N, 256] f32, conv_w_*: [C=2, E, 1, 1] f32,
  gcn_weight: [256, 64] f32 -> out [N, C*64] f32.

Math (per channel c):
  a = sum_e softmax(w1)[c,e] A[e];  b, a1 likewise with w2, w3
  H0 = a @ b
  H0n = H0 * 1/(colsum(H0)+eps)          (norm add=False; diag term dropped,
                                          verified 3.8e-4 rel err in fp64)
  H1 = H0n @ a1
  H1d = H1 with diag set to 1            (norm add=True diag handling is
                                          mandatory: without it 4.1e-2)
  out_c = relu(H1d^T @ (X @ W) * 1/(colsum(H1d)+eps)[:,None])

Sharding: channel-split. Cores 0-3 = channel 0, cores 4-7 = channel 1;
within a group, 512-row shards (core c: rows 512*(c%4)...). bf16 compute,
fp32 PSUM. Collectives:
  C1: four 8-core Shared-output AllGather chunks carrying b-shard halves
      (+ colsum(a) partials) and a1-shard halves.
  C2: one 4-core-group AllReduce of readout partials [2048, 65]
      (64 feature cols + colsum(H1d) in col 64).
"""
import sys
import types

import numpy as np
import ml_dtypes

P = 128
N = 2048
S = 512            # shard rows per core
E = 5
TK = N // P        # 16 k tiles
TI = S // P        # 4 i tiles
Q = 512            # mm column-quarter width
EPS = 1e-8
GROUPS8 = [[0, 1, 2, 3, 4, 5, 6, 7]]
GROUPS4 = [[0, 1, 2, 3], [4, 5, 6, 7]]

_nc_cache = None


def _install_ntff_hook():
    if "antenv.axon_hooks" in sys.modules:
        return
    try:
        from trn_agent_boot.trn_boot import _ntff_profile_via_ctypes
        hook = _ntff_profile_via_ctypes("/opt/axon/libaxon_pjrt.so")
    except Exception:
        hook = None
    mod = types.ModuleType("antenv.axon_hooks")
    mod.get_axon_ntff_profile_hook = lambda: hook
    mod.set_axon_ntff_profile_hook = lambda h: None
    sys.modules["antenv.axon_hooks"] = mod


def _build_nc():
    import concourse.mybir as mybir
    import concourse.tile as tile
    from concourse import bacc
    from concourse.bass import ds
    from concourse.masks import make_identity

    bf16 = mybir.dt.bfloat16
    f32 = mybir.dt.float32
    Alu = mybir.AluOpType

    nc = bacc.Bacc(None)
    nc.num_devices = 8

    a_rows = nc.dram_tensor("a_rows", [E, S, N], bf16, kind="ExternalInput")
    s1 = nc.dram_tensor("s1", [P, E], f32, kind="ExternalInput")
    s2 = nc.dram_tensor("s2", [P, E], f32, kind="ExternalInput")
    s3 = nc.dram_tensor("s3", [P, E], f32, kind="ExternalInput")
    xt = nc.dram_tensor("xt", [256, S], bf16, kind="ExternalInput")
    w_in = nc.dram_tensor("w", [256, 64], bf16, kind="ExternalInput")
    doff = nc.dram_tensor("doff", [P, TI], f32, kind="ExternalInput")
    out = nc.dram_tensor("out", [N, 64], f32, kind="ExternalOutput")

    with tile.TileContext(nc) as tc:
        with (
            tc.tile_pool(name="pers", bufs=1) as pers,
            tc.tile_pool(name="work", bufs=3) as work,
            tc.tile_pool(name="pan", bufs=6) as panp,
            tc.tile_pool(name="ps", bufs=8, space="PSUM") as psp,
            tc.tile_pool(name="dram", bufs=1, space="DRAM") as dram,
        ):
            pid = nc.partition_id()
            g4_514 = (pid // 4) * (4 * 514)   # C1a block base (514 rows/rank)
            g4_512 = (pid // 4) * (4 * 512)   # C1b/c/d block base

            # ---- small SBUF constants ----
            s1_sb = pers.tile([P, E], f32, name="s1_sb")
            s2_sb = pers.tile([P, E], f32, name="s2_sb")
            s3_sb = pers.tile([P, E], f32, name="s3_sb")
            nc.sync.dma_start(s1_sb[:], s1[:])
            nc.sync.dma_start(s2_sb[:], s2[:])
            nc.sync.dma_start(s3_sb[:], s3[:])
            doff_sb = pers.tile([P, TI], f32, name="doff_sb")
            nc.sync.dma_start(doff_sb[:], doff[:])
            ident = pers.tile([P, P], bf16, name="ident")
            make_identity(nc, ident)
            ones_col = pers.tile([P, 1], bf16, name="ones_col")
            nc.gpsimd.memset(ones_col[:], 1.0)

            # ---- C1 AllGather buffers (8-core, Shared outputs) ----
            c1a_in = dram.tile([514, 1024], bf16, name="c1a_in")
            c1b_in = dram.tile([512, 1024], bf16, name="c1b_in")
            c1c_in = dram.tile([512, 1024], bf16, name="c1c_in")
            c1d_in = dram.tile([512, 1024], bf16, name="c1d_in")
            c1a_out = dram.tile([8 * 514, 1024], bf16, name="c1a_out",
                                addr_space="Shared")
            c1b_out = dram.tile([8 * 512, 1024], bf16, name="c1b_out",
                                addr_space="Shared")
            c1c_out = dram.tile([8 * 512, 1024], bf16, name="c1c_out",
                                addr_space="Shared")
            c1d_out = dram.tile([8 * 512, 1024], bf16, name="c1d_out",
                                addr_space="Shared")

            def ag(inp, outp):
                nc.gpsimd.collective_compute(
                    "AllGather", Alu.bypass, replica_groups=GROUPS8,
                    ins=[inp.opt()], outs=[outp.opt()])

            # warm-up: absorb the first-collective setup cost during convs
            wu_in = dram.tile([8, 128], bf16, name="wu_in")
            wu_out = dram.tile([64, 128], bf16, name="wu_out", addr_space="Shared")
            wu_sb = work.tile([8, 128], bf16, name="wu_sb", bufs=1)
            nc.vector.memset(wu_sb[:], 0.0)
            nc.sync.dma_start(wu_in[:], wu_sb[:])
            ag(wu_in, wu_out)

            # ---- pass 1: conv b per row-tile, C1a/C1b launch ASAP;
            #      pass 2: conv a (+ca partials, aT transposes) -> C1 riders;
            #      pass 3: conv a1 -> C1c/C1d. A rows re-streamed per pass so
            #      the b-shard AllGather starts ~100us earlier. Conv tiles are
            #      split DVE/GpSimd (independent accumulation chains). ----
            aT = [pers.tile([P, S], bf16, name=f"aT_{k}") for k in range(TK)]
            ca_sb = pers.tile([1, N], bf16, name="ca_sb")
            ca_ps = [psp.tile([1, Q], f32, name=f"ca_ps_{cb}", tag="ps")
                     for cb in range(4)]

            def conv_tile(eng, dst, s_ap, Ats):
                eng.tensor_scalar(
                    dst[:], Ats[0][:], s_ap[:, 0:1], None, op0=Alu.mult)
                for e in range(1, E):
                    eng.scalar_tensor_tensor(
                        dst[:], Ats[e][:], s_ap[:, e:e + 1], dst[:],
                        op0=Alu.mult, op1=Alu.add)

            with tc.tile_pool(name="apool", bufs=2) as apool:
                def a_pass(body):
                    for t in range(TI):
                        Ats = [apool.tile([P, N], bf16, name=f"At{e}",
                                          tag=f"At{e}") for e in range(E)]
                        for e in range(E):
                            nc.sync.dma_start(Ats[e][:],
                                              a_rows[e, P * t:P * (t + 1), :])
                        eng = nc.vector
                        body(t, Ats, eng)

                def pass_b(t, Ats, eng):
                    bt = apool.tile([P, N], bf16, name="bt", tag="bt")
                    conv_tile(eng, bt, s2_sb, Ats)
                    nc.sync.dma_start(c1a_in[P * t:P * (t + 1), :], bt[:, 0:1024])
                    nc.sync.dma_start(c1b_in[P * t:P * (t + 1), :],
                                      bt[:, 1024:2048])
                a_pass(pass_b)
                ag(c1b_in, c1b_out)

                def pass_a1(t, Ats, eng):
                    a1t = apool.tile([P, N], bf16, name="a1t", tag="a1t")
                    conv_tile(eng, a1t, s3_sb, Ats)
                    nc.sync.dma_start(c1c_in[P * t:P * (t + 1), :], a1t[:, 0:1024])
                    nc.sync.dma_start(c1d_in[P * t:P * (t + 1), :],
                                      a1t[:, 1024:2048])
                a_pass(pass_a1)
                ag(c1c_in, c1c_out)

                def pass_a(t, Ats, eng):
                    at = apool.tile([P, N], bf16, name="at", tag="at")
                    conv_tile(eng, at, s1_sb, Ats)
                    for cb in range(4):
                        nc.tensor.matmul(
                            ca_ps[cb][:], ones_col[:], at[:, Q * cb:Q * (cb + 1)],
                            start=(t == 0), stop=(t == TI - 1))
                    for k in range(TK):
                        pt = psp.tile([P, P], bf16, name="ptt", tag="ps")
                        nc.tensor.transpose(pt[:], at[:, P * k:P * (k + 1)],
                                            ident[:])
                        nc.vector.tensor_copy(aT[k][:, P * t:P * (t + 1)], pt[:])
                a_pass(pass_a)
                for cb in range(4):
                    nc.vector.tensor_copy(ca_sb[0:1, Q * cb:Q * (cb + 1)],
                                          ca_ps[cb][:])
                nc.sync.dma_start(c1a_in[512:513, :], ca_sb[0:1, 0:1024])
                nc.sync.dma_start(c1a_in[513:514, :], ca_sb[0:1, 1024:2048])
                ag(c1a_in, c1a_out)
                ag(c1d_in, c1d_out)

            # ---- during C1: X@W, iota masks ----
            xt_sb = [pers.tile([P, S], bf16, name=f"xt_{k}") for k in range(2)]
            w_sb = [pers.tile([P, 64], bf16, name=f"w_{k}") for k in range(2)]
            for k in range(2):
                nc.sync.dma_start(xt_sb[k][:], xt[P * k:P * (k + 1), :])
                nc.sync.dma_start(w_sb[k][:], w_in[P * k:P * (k + 1), :])
            xwo = [pers.tile([P, 65], bf16, name=f"xwo_{t}") for t in range(TI)]
            for t in range(TI):
                px = psp.tile([P, 64], f32, name="px", tag="ps")
                for k in range(2):
                    nc.tensor.matmul(px[:], xt_sb[k][:, P * t:P * (t + 1)], w_sb[k][:],
                                     start=(k == 0), stop=(k == 1))
                nc.vector.tensor_copy(xwo[t][:, 0:64], px[:])
                nc.gpsimd.memset(xwo[t][:, 64:65], 1.0)

            u8 = mybir.dt.uint8
            masks = [pers.tile([P, N], u8, name=f"mask_{t}") for t in range(TI)]
            ones_t = pers.tile([P, N], bf16, name="ones_t")
            nc.gpsimd.memset(ones_t[:], 1.0)
            with tc.tile_pool(name="iotap", bufs=1) as iotap:
                iota_f = iotap.tile([P, N], f32, name="iota_f")
                nc.gpsimd.iota(iota_f[:], pattern=[[1, N]], base=0,
                               channel_multiplier=-1,
                               allow_small_or_imprecise_dtypes=True)
                for t in range(TI):
                    nc.vector.tensor_scalar(
                        masks[t][:], iota_f[:], doff_sb[:, t:t + 1], None,
                        op0=Alu.is_equal)

            # ---- ca_full: sum own group's 4 partials from c1a_out ----
            ca_full = work.tile([1, N], f32, name="ca_full", bufs=1)
            cp = []
            for r in range(4):
                off = g4_514 + r * 514 + 512
                cpr = work.tile([1, N], bf16, name="cpr", tag="cpr", bufs=2)
                nc.sync.dma_start(cpr[0:1, 0:1024], c1a_out[ds(off, 1), :])
                nc.sync.dma_start(cpr[0:1, 1024:N], c1a_out[ds(off + 1, 1), :])
                cp.append(cpr)
                if r == 1:
                    nc.vector.tensor_add(ca_full[:], cp[0][:], cp[1][:])
                elif r > 1:
                    nc.vector.tensor_add(ca_full[:], ca_full[:], cpr[:])
            ca_d = dram.tile([1, N], f32, name="ca_d")
            nc.sync.dma_start(ca_d[:], ca_full[:])
            caTb = work.tile([P, TK], bf16, name="caTb", bufs=1)
            nc.gpsimd.dma_start(caTb[:],
                                ca_d[0:1, :].rearrange("a (t p) -> (a p) t", p=P))

            # ---- mm1: H0[R,:] = a[R,:] @ b  (+ deg0 = ca @ b) ----
            H0 = [pers.tile([P, N], bf16, name=f"H0_{t}") for t in range(TI)]
            deg0 = work.tile([1, N], f32, name="deg0", bufs=1)
            for q in (2, 3, 0, 1):
                half_out = c1a_out if q < 2 else c1b_out
                col0 = (q % 2) * 512
                shard_rows = 514 if q < 2 else 512
                base = g4_514 if q < 2 else g4_512
                pts = [psp.tile([P, Q], f32, name=f"pt1_{i}", tag="ps")
                       for i in range(TI)]
                for k in range(TK):
                    pan = panp.tile([P, Q], bf16, name="pan")
                    off = base + (k // 4) * shard_rows + (k % 4) * P
                    nc.sync.dma_start(pan[:], half_out[ds(off, P), col0:col0 + Q])
                    for i in range(TI):
                        nc.tensor.matmul(pts[i][:], aT[k][:, P * i:P * (i + 1)],
                                         pan[:], start=(k == 0), stop=(k == TK - 1))
                for i in range(TI):
                    nc.vector.tensor_copy(H0[i][:, Q * q:Q * (q + 1)], pts[i][:])

            # ---- deg0 = ca @ b: dedicated panel pass (decoupled from mm1) ----
            ptds = [psp.tile([1, Q], f32, name=f"ptd_{qq}", tag="ps")
                    for qq in range(4)]
            for k in range(TK):
                pand = panp.tile([P, N], bf16, name="pand", tag="pand", bufs=3)
                offa = g4_514 + (k // 4) * 514 + (k % 4) * P
                offb = g4_512 + (k // 4) * 512 + (k % 4) * P
                nc.sync.dma_start(pand[:, 0:1024], c1a_out[ds(offa, P), :])
                nc.sync.dma_start(pand[:, 1024:2048], c1b_out[ds(offb, P), :])
                for qq in range(4):
                    nc.tensor.matmul(ptds[qq][:], caTb[:, k:k + 1],
                                     pand[:, Q * qq:Q * (qq + 1)],
                                     start=(k == 0), stop=(k == TK - 1))
            for qq in range(4):
                nc.vector.tensor_copy(deg0[0:1, Q * qq:Q * (qq + 1)], ptds[qq][:])

            # deginv0, bounced to per-partition [P, TK] layout
            nc.vector.tensor_scalar(deg0[:], deg0[:], float(EPS), None, op0=Alu.add)
            nc.vector.reciprocal(deg0[:], deg0[:])
            dinv0_d = dram.tile([1, N], f32, name="dinv0_d")
            nc.sync.dma_start(dinv0_d[:], deg0[:])
            dinv0T = work.tile([P, TK], f32, name="dinv0T", bufs=1)
            nc.sync.dma_start(dinv0T[:],
                              dinv0_d[0:1, :].rearrange("a (t p) -> (a p) t", p=P))

            # ---- lhsT for mm2: (H0^T) * deginv0[k], bf16 ----
            l0 = [pers.tile([P, S], bf16, name=f"l0_{k}") for k in range(TK)]
            for k in range(TK):
                for t in range(TI):
                    pt = psp.tile([P, P], bf16, name="ptt2", tag="ps")
                    nc.tensor.transpose(pt[:], H0[t][:, P * k:P * (k + 1)], ident[:])
                    nc.vector.tensor_scalar(
                        l0[k][:, P * t:P * (t + 1)], pt[:], dinv0T[:, k:k + 1], None,
                        op0=Alu.mult)

            # ---- mm2: H1[R,:] = H0n[R,:] @ a1 ----
            H1 = [pers.tile([P, N], bf16, name=f"H1_{t}") for t in range(TI)]
            for q in range(4):
                half_out = c1c_out if q < 2 else c1d_out
                col0 = (q % 2) * 512
                pts = [psp.tile([P, Q], f32, name=f"pt2_{i}", tag="ps")
                       for i in range(TI)]
                for k in range(TK):
                    pan = panp.tile([P, Q], bf16, name="pan2")
                    off = g4_512 + (k // 4) * 512 + (k % 4) * P
                    nc.sync.dma_start(pan[:], half_out[ds(off, P), col0:col0 + Q])
                    for i in range(TI):
                        nc.tensor.matmul(pts[i][:], l0[k][:, P * i:P * (i + 1)],
                                         pan[:], start=(k == 0), stop=(k == TK - 1))
                for i in range(TI):
                    nc.vector.tensor_copy(H1[i][:, Q * q:Q * (q + 1)], pts[i][:])

            # ---- norm1 diag: H1[i, 512r+128t+i] <- 1.0 ----
            for t in range(TI):
                nc.vector.copy_predicated(H1[t][:], masks[t][:], ones_t[:])

            # ---- readout partials + colsum(H1d): [N, 65] ----
            c2_in = dram.tile([N, 65], f32, name="c2_in")
            c2_out = dram.tile([N, 65], f32, name="c2_out")
            for j in range(TK):
                pr = psp.tile([P, 65], f32, name="pr", tag="ps")
                for i in range(TI):
                    nc.tensor.matmul(pr[:], H1[i][:, P * j:P * (j + 1)], xwo[i][:],
                                     start=(i == 0), stop=(i == TI - 1))
                ro = work.tile([P, 65], f32, name="ro")
                nc.vector.tensor_copy(ro[:], pr[:])
                nc.sync.dma_start(c2_in[P * j:P * (j + 1), :], ro[:])
            nc.gpsimd.collective_compute(
                "AllReduce", Alu.add, replica_groups=GROUPS4,
                ins=[c2_in.opt()], outs=[c2_out.opt()])

            # ---- final: relu(partial * deginv1) ----
            fo = work.tile([P, TK * 65], f32, name="fo", bufs=1)
            for j in range(TK):
                nc.sync.dma_start(fo[:, j * 65:(j + 1) * 65],
                                  c2_out[P * j:P * (j + 1), :])
            dinv1 = work.tile([P, TK], f32, name="dinv1", bufs=1)
            nc.vector.tensor_scalar(
                dinv1[:], fo[:, 64::65], float(EPS), None, op0=Alu.add)
            nc.vector.reciprocal(dinv1[:], dinv1[:])
            for j in range(TK):
                oj = work.tile([P, 64], f32, name="oj")
                nc.vector.tensor_scalar(oj[:], fo[:, j * 65:j * 65 + 64],
                                        dinv1[:, j:j + 1], 0.0,
                                        op0=Alu.mult, op1=Alu.max)
                nc.sync.dma_start(out[P * j:P * (j + 1), :], oj[:])

    nc.finalize()
    return nc


def _get_nc():
    global _nc_cache
    if _nc_cache is None:
        _nc_cache = _build_nc()
    return _nc_cache


def _softmax(w):
    m = w.max(axis=1, keepdims=True)
    e = np.exp(w - m)
    return e / e.sum(axis=1, keepdims=True)


def _run(A, X, conv_w_l0_1, conv_w_l0_2, conv_w_l1, gcn_weight, trace=False):
    _install_ntff_hook()
    from concourse.bass_utils import run_bass_kernel_spmd

    bf16 = ml_dtypes.bfloat16
    A = np.ascontiguousarray(np.asarray(A, np.float32)).astype(bf16)
    X = np.asarray(X, np.float32)
    s1 = _softmax(np.asarray(conv_w_l0_1, np.float32)[:, :, 0, 0])  # [2, 5]
    s2 = _softmax(np.asarray(conv_w_l0_2, np.float32)[:, :, 0, 0])
    s3 = _softmax(np.asarray(conv_w_l1, np.float32)[:, :, 0, 0])
    w = np.ascontiguousarray(np.asarray(gcn_weight, np.float32)).astype(bf16)

    in_maps = []
    for c in range(8):
        r, g = c % 4, c // 4
        rows = slice(S * r, S * (r + 1))
        in_maps.append({
            "a_rows": np.ascontiguousarray(A[:, rows, :]),
            "s1": np.ascontiguousarray(np.broadcast_to(s1[g], (P, E))).astype(np.float32),
            "s2": np.ascontiguousarray(np.broadcast_to(s2[g], (P, E))).astype(np.float32),
            "s3": np.ascontiguousarray(np.broadcast_to(s3[g], (P, E))).astype(np.float32),
            "xt": np.ascontiguousarray(X[rows, :].T.astype(bf16)),
            "w": w,
            "doff": np.ascontiguousarray(np.broadcast_to(
                (S * r + P * np.arange(4, dtype=np.float32))[None, :],
                (P, 4))).astype(np.float32),
        })

    nc = _get_nc()
    res = run_bass_kernel_spmd(nc, in_maps, core_ids=list(range(8)), trace=trace)
    out = np.concatenate([res.results[0]["out"], res.results[4]["out"]], axis=1)
    return np.ascontiguousarray(out.astype(np.float32)), res


def kernel(A, X, conv_w_l0_1, conv_w_l0_2, conv_w_l1, gcn_weight):
    out, _ = _run(A, X, conv_w_l0_1, conv_w_l0_2, conv_w_l1, gcn_weight)
    return out
